# revision 29
# baseline (speedup 1.0000x reference)
"""Trainium2 Bass kernel for the ragged Expand op (nn_Expand_24386824307320).

Semantics (matches the TF Expand layer / jax reference):
  x          [16, 4096, 256] f32
  dimensions [16, 4096, 1]   int32 repeat counts in [0, 8)
  out        [16, T, 256]    f32 where T = max_b sum_s d[b,s]
  out[b, t]  = x[b, idx[b,t]] for t < totals[b] else 0, with
  idx[b, t]  = searchsorted(cumsum(d[b]), t, side='right')

Strategy: pure batch data-parallel over 8 NeuronCores (2 examples/core).

The expansion happens ON-CHIP via PE matmul with an on-chip-generated 0/1
selection matrix, so HBM traffic is ~35 MB/core (read x once as bf16 +
write out once) instead of the ~59 MB/core of an HBM-source row gather:

  Pool: partition_broadcast(idxrel row)            -> [W1, 256] bf16
  DVE:  is_equal(bcast, per-partition iota)        -> G bf16
  PE:   out_tile[128,256] = G[:, m-tile].T @ x_blk   (one matmul per tile)
  Act:  x/idx loads (its own HWDGE ring) + one [128, 512] PSUM->SBUF
        copy per block
  SP:   writes only (other HWDGE ring), two blocks per DMA

Out rows are processed in uniform 256-row blocks (partition p of a block
holds rows 2p, 2p+1), making the program identical across cores/examples
(pure SPMD; all data-dependence lives in host-built input tensors). Each
block's 256 rows span <= W1 source rows (W1 from the data, ~96; a second
K-window W2 is emitted only if some block spans beyond 128). Keeping the
write ring free of loads lets the output stream start as soon as the
first block drains (~13us). bf16 rounding of x gives rel err ~1.6e-3,
well under the 2e-2 gate.
"""

import numpy as np

B, S, D = 16, 4096, 256
NCORES = 8
EX_PER_CORE = B // NCORES  # 2
NK = 256  # out rows per block
CPP = NK // 128  # out rows per partition per block = 2

# pipeline buffer depths
BCB = 6  # bcast sbuf bufs
GN = 8  # G sbuf bufs
PSB = 8  # psum block tensors [128, CPP, D] (1 bank each)
SNP = 6  # staging slot-pairs (12 block slots)
XCH = 8  # x load chunks


def _plan(dimensions):
    d = dimensions[:, :, 0].astype(np.int64)  # [B,S]
    totals = d.sum(1)  # [B]
    T = int(totals.max())
    csum = d.cumsum(1)  # [B,S]
    pos = np.arange(T)
    idx = np.empty((B, T), np.int64)
    for b in range(B):
        idx[b] = np.searchsorted(csum[b], pos, side="right")
    idx = np.minimum(idx, S - 1)
    return T, idx, totals


def build_program(NBE, W1, W2, nk_last, ld_chunk):
    """One SPMD program: NBE NK-row blocks per example, 2 examples.
    ld_chunk[bi] = x-chunk index of block bi. W2=0 means every block's
    source span fits the single W1-row K-window.

    Write units: per example, consecutive block pairs (blk 2j, 2j+1) go out
    as one DMA; an odd final block goes out alone with the nk_last trim.
    Unit u uses staging slot-pair u%SNP (stg block slots 2*(u%SNP)+{0,1}).
    """
    import concourse.bass as bass
    import concourse.bacc as bacc
    import concourse.mybir as mybir
    from concourse import library_config
    from contextlib import ExitStack

    NBLK = EX_PER_CORE * NBE
    R_ex = NBE * NK

    # write units: (blocks, is_last)
    units = []
    for e in range(EX_PER_CORE):
        blk = 0
        while blk < NBE:
            if blk + 1 < NBE:
                units.append((e * NBE + blk, 2))
                blk += 2
            else:
                units.append((e * NBE + blk, 1))
                blk += 1
    unit_of = {}
    slot_of = {}
    for u, (b0, n) in enumerate(units):
        for k in range(n):
            unit_of[b0 + k] = u
            slot_of[b0 + k] = 2 * (u % SNP) + k

    nc = bacc.Bacc("TRN2", num_devices=NCORES, name="expand_pe")
    xw1_t = nc.dram_tensor("xw1", [W1, NBLK, D], mybir.dt.bfloat16, kind="ExternalInput")
    if W2:
        xw2_t = nc.dram_tensor("xw2", [W2, NBLK, D], mybir.dt.bfloat16, kind="ExternalInput")
    idxr_t = nc.dram_tensor("idxr", [1, NBLK, NK], mybir.dt.bfloat16, kind="ExternalInput")
    out_t = nc.dram_tensor(
        "out", [EX_PER_CORE * R_ex, D], mybir.dt.float32, kind="ExternalOutput"
    )

    xsplit = [(i * NBLK) // XCH for i in range(XCH + 1)]

    with (
        nc.sbuf_tensor("xw1_sb", [W1, NBLK, D], mybir.dt.bfloat16) as xw1_sb,
        nc.sbuf_tensor("idxr_sb", [1, NBLK, NK], mybir.dt.bfloat16) as idxr_sb,
        nc.sbuf_tensor("iota_sb", [128, 2], mybir.dt.float32) as iota_sb,
        nc.sbuf_tensor("bcb", [128, BCB, NK], mybir.dt.bfloat16) as bcb,
        nc.sbuf_tensor("gb1", [W1, GN, NK], mybir.dt.bfloat16) as gb1,
        nc.sbuf_tensor("stg", [128, 2 * SNP, CPP, D], mybir.dt.float32) as stg,
        nc.semaphore("ldc") as ldc,
        nc.semaphore("ldc2") as ldc2,
        nc.semaphore("bc") as bc,
        nc.semaphore("cmp") as cmp_s,
        nc.semaphore("mm") as mm,
        nc.semaphore("dr") as dr,
        ExitStack() as stack,
    ):
        if W2:
            xw2_sb = stack.enter_context(
                nc.sbuf_tensor("xw2_sb", [W2, NBLK, D], mybir.dt.bfloat16)
            )
            gb2 = stack.enter_context(
                nc.sbuf_tensor("gb2", [W2, GN, NK], mybir.dt.bfloat16)
            )
        ldx = [stack.enter_context(nc.semaphore(f"ldx{c}")) for c in range(XCH)]  # noqa: ANT232
        wsl = [stack.enter_context(nc.semaphore(f"wsl{s}")) for s in range(SNP)]  # noqa: ANT232
        pst = [
            stack.enter_context(  # noqa: ANT232
                nc.psum_tensor(f"ps{t}", [128, CPP, D], mybir.dt.float32)
            )
            for t in range(PSB)
        ]
        block = stack.enter_context(nc.Block())

        @block.sync
        def _(sy):
            # idxr rides this ring (empty until the first write ~15us);
            # small head first so Pool can start broadcasting early
            sy.dma_start(idxr_sb[:, :16, :], idxr_t.ap()[:, :16, :]).then_inc(ldc, 16)
            sy.dma_start(idxr_sb[:, 16:, :], idxr_t.ap()[:, 16:, :]).then_inc(ldc2, 16)
            for u, (b0, n) in enumerate(units):
                sy.wait_ge(dr, b0 + n)
                r0 = b0 * NK
                if n == 2:
                    dst = out_t.ap()[r0 : r0 + 2 * NK, :].rearrange(
                        "(b p c) e -> p b c e", b=2, p=128
                    )
                    src = stg[:, 2 * (u % SNP) : 2 * (u % SNP) + 2, :, :]
                else:
                    nk = nk_last
                    dst = out_t.ap()[r0 : r0 + nk, :].rearrange(
                        "(p c) e -> p c e", p=nk // CPP
                    )
                    src = stg[: nk // CPP, 2 * (u % SNP), :, :]
                sy.dma_start(dst, src).then_inc(wsl[u % SNP], 16)
            for s in range(SNP):
                nwr = len(units) // SNP + (1 if s < len(units) % SNP else 0)
                sy.wait_ge(wsl[s], 16 * nwr)

        @block.scalar
        def _(sc):
            lo, hi = xsplit[0], xsplit[1]
            sc.dma_start(xw1_sb[:, lo:hi, :], xw1_t.ap()[:, lo:hi, :]).then_inc(ldx[0], 16)
            if W2:
                sc.dma_start(xw2_sb[:, lo:hi, :], xw2_t.ap()[:, lo:hi, :]).then_inc(ldx[0], 16)
            for bi in range(NBLK):
                if bi >= 4 and (bi - 4) % 8 == 0 and (bi - 4) // 8 + 1 < XCH:
                    c = (bi - 4) // 8 + 1
                    clo, chi = xsplit[c], xsplit[c + 1]
                    sc.dma_start(
                        xw1_sb[:, clo:chi, :], xw1_t.ap()[:, clo:chi, :]
                    ).then_inc(ldx[c], 16)
                    if W2:
                        sc.dma_start(
                            xw2_sb[:, clo:chi, :], xw2_t.ap()[:, clo:chi, :]
                        ).then_inc(ldx[c], 16)
                sc.wait_ge(mm, bi + 1)
                u = unit_of[bi]
                if u >= SNP and slot_of[bi] == 2 * (u % SNP):
                    sc.wait_ge(wsl[u % SNP], 16 * (u // SNP))
                sc.copy(stg[:, slot_of[bi], :, :], pst[bi % PSB][:, :, :]).then_inc(
                    dr, 1
                )

        @block.gpsimd
        def _(gp):
            # iota values 0..255 are exact in f32
            gp.iota(
                iota_sb[:],
                [[128, 2]],
                base=0,
                channel_multiplier=1,
                allow_small_or_imprecise_dtypes=True,
            )
            gp.load_library(library_config.mlp)
            gp.wait_ge(ldc, 16)
            for bi in range(NBLK):
                if bi == 16:
                    gp.wait_ge(ldc2, 16)
                if bi >= BCB:
                    gp.wait_ge(cmp_s, bi - BCB + 1)
                gp.partition_broadcast(
                    bcb[: max(W1, W2), bi % BCB, :], idxr_sb[:1, bi, :]
                ).then_inc(bc, 1)

        @block.vector
        def _(ve):
            for bi in range(NBLK):
                ve.wait_ge(bc, bi + 1)
                if bi >= GN:
                    ve.wait_ge(mm, bi - GN + 1)
                c1 = ve.tensor_scalar(
                    gb1[:, bi % GN, :],
                    bcb[:W1, bi % BCB, :],
                    iota_sb[:W1, :1],
                    None,
                    mybir.AluOpType.is_equal,
                )
                if W2:
                    ve.tensor_scalar(
                        gb2[:, bi % GN, :],
                        bcb[:W2, bi % BCB, :],
                        iota_sb[:W2, 1:2],
                        None,
                        mybir.AluOpType.is_equal,
                    ).then_inc(cmp_s, 1)
                else:
                    c1.then_inc(cmp_s, 1)

        @block.tensor
        def _(te):
            for bi in range(NBLK):
                te.wait_ge(cmp_s, bi + 1)
                if bi == 0 or ld_chunk[bi] != ld_chunk[bi - 1]:
                    te.wait_ge(ldx[ld_chunk[bi]], 32 if W2 else 16)
                if bi >= PSB:
                    te.wait_ge(dr, bi - PSB + 1)
                for m in range(CPP):
                    mm1 = te.matmul(
                        pst[bi % PSB][:, m, :],
                        gb1[:, bi % GN, m * 128 : (m + 1) * 128],
                        xw1_sb[:, bi, :],
                        start=True,
                        stop=not W2,
                    )
                    if W2:
                        mm1 = te.matmul(
                            pst[bi % PSB][:, m, :],
                            gb2[:, bi % GN, m * 128 : (m + 1) * 128],
                            xw2_sb[:, bi, :],
                            start=False,
                            stop=True,
                        )
                    if m == CPP - 1:
                        mm1.then_inc(mm, 1)

    nc.compile()
    return nc


def _install_ntff_hook():
    """Provide the antenv.axon_hooks module bass_utils expects for NTFF
    tracing under axon (the agent image ships without it)."""
    import sys
    import types

    if "antenv.axon_hooks" in sys.modules:
        return
    from trn_agent_boot.trn_boot import _ntff_profile_via_ctypes

    hook = _ntff_profile_via_ctypes("/opt/axon/libaxon_pjrt.so")
    mod = types.ModuleType("antenv.axon_hooks")
    state = {"hook": hook}
    mod.get_axon_ntff_profile_hook = lambda: state["hook"]
    mod.set_axon_ntff_profile_hook = lambda h: state.update(hook=h)
    sys.modules["antenv.axon_hooks"] = mod


def kernel(x, dimensions, _trace=False, _sim_core=None):
    import ml_dtypes

    x = np.ascontiguousarray(np.asarray(x), dtype=np.float32)
    dimensions = np.asarray(dimensions).astype(np.int32)

    T, idx, totals = _plan(dimensions)
    NBE = (T + NK - 1) // NK
    R_ex = NBE * NK
    NBLK = EX_PER_CORE * NBE
    nk_last = ((T - (NBE - 1) * NK + CPP - 1) // CPP) * CPP

    # K-window sizes from the data (uniform across the whole batch)
    max_span = 0
    for bb in range(B):
        tot = int(totals[bb])
        for blk in range(NBE):
            c0 = blk * NK
            c1 = min((blk + 1) * NK, tot, T)
            if c0 < c1:
                max_span = max(max_span, int(idx[bb, c1 - 1] - idx[bb, c0] + 1))
    # full 128 rows so x loads engage all 16 DMA engines at line rate
    W1 = 128
    W2 = max(32, ((max_span - 128 + 31) // 32) * 32) if max_span > 128 else 0
    assert max_span <= 128 + (W2 or 0), f"block span {max_span} exceeds 128+W2"

    xbf = x.astype(ml_dtypes.bfloat16)
    xw1 = np.zeros((B, W1, NBE, D), ml_dtypes.bfloat16)
    xw2 = np.zeros((B, W2, NBE, D), ml_dtypes.bfloat16) if W2 else None
    idxr = np.full((B, NBE, NK), -1.0, np.float32)
    for bb in range(B):
        tot = int(totals[bb])
        for blk in range(NBE):
            c0 = blk * NK
            c1 = min((blk + 1) * NK, tot)
            s = int(idx[bb, c0]) if c0 < c1 else 0
            n1 = min(W1, S - s)
            xw1[bb, :n1, blk] = xbf[bb, s : s + n1]
            if W2:
                n2 = min(W2, S - (s + 128))
                if n2 > 0:
                    xw2[bb, :n2, blk] = xbf[bb, s + 128 : s + 128 + n2]
            if c0 >= c1:
                continue
            t = np.arange(c0, c1)
            q = t - c0
            cols = (q % CPP) * 128 + (q // CPP)
            idxr[bb, blk, cols] = idx[bb, t] - s
    idxr_bf = idxr.astype(ml_dtypes.bfloat16)

    xsplit = [(i * NBLK) // XCH for i in range(XCH + 1)]
    ld_chunk = [
        next(ci for ci in range(XCH) if bi < xsplit[ci + 1]) for bi in range(NBLK)
    ]

    in_maps = []
    for core in range(NCORES):
        exs = [EX_PER_CORE * core + e for e in range(EX_PER_CORE)]
        im = {
            "xw1": np.concatenate([xw1[bb] for bb in exs], axis=1),
            "idxr": np.concatenate([idxr_bf[bb] for bb in exs], axis=0)[None],
        }
        if W2:
            im["xw2"] = np.concatenate([xw2[bb] for bb in exs], axis=1)
        in_maps.append(im)

    nc = build_program(NBE, W1, W2, nk_last, ld_chunk)

    if _sim_core is not None:
        import concourse.bass_interp as bass_interp

        sim = bass_interp.CoreSim(nc)
        for k, v in in_maps[_sim_core].items():
            sim.tensor(k)[:] = v
        sim.simulate()
        st = np.asarray(sim.tensor("out"), dtype=np.float32)
        out = np.empty((EX_PER_CORE, T, D), np.float32)
        for e in range(EX_PER_CORE):
            out[e] = st[e * R_ex : e * R_ex + T]
        return out

    import concourse.bass_utils as bass_utils

    if _trace:
        _install_ntff_hook()
        bass_utils.upload_artifacts = lambda tmpdir: tmpdir

    res = bass_utils.run_bass_kernel_spmd(
        nc, in_maps, core_ids=list(range(NCORES)), trace=_trace
    )

    out = np.empty((B, T, D), np.float32)
    for core in range(NCORES):
        st = res.results[core]["out"]
        for e in range(EX_PER_CORE):
            out[EX_PER_CORE * core + e] = st[e * R_ex : e * R_ex + T]
    if _trace:
        kernel.last_results = res
    return out


# revision 30
# speedup vs baseline: 1.0409x; 1.0409x over previous
"""Trainium2 Bass kernel for the ragged Expand op (nn_Expand_24386824307320).

Semantics (matches the TF Expand layer / jax reference):
  x          [16, 4096, 256] f32
  dimensions [16, 4096, 1]   int32 repeat counts in [0, 8)
  out        [16, T, 256]    f32 where T = max_b sum_s d[b,s]
  out[b, t]  = x[b, idx[b,t]] for t < totals[b] else 0, with
  idx[b, t]  = searchsorted(cumsum(d[b]), t, side='right')

Strategy: pure batch data-parallel over 8 NeuronCores (2 examples/core).

The expansion happens ON-CHIP via PE matmul with an on-chip-generated 0/1
selection matrix, so HBM traffic is ~35 MB/core (read x once as bf16 +
write out once) instead of the ~59 MB/core of an HBM-source row gather:

  Pool: partition_broadcast(idxrel row)            -> [W1, 256] bf16
  DVE:  is_equal(bcast, per-partition iota)        -> G bf16
  PE:   out_tile[128,256] = G[:, m-tile].T @ x_blk   (one matmul per tile)
  Act:  x/idx loads (its own HWDGE ring) + one [128, 512] PSUM->SBUF
        copy per block
  SP:   writes only (other HWDGE ring), two blocks per DMA

Out rows are processed in uniform 256-row blocks (partition p of a block
holds rows 2p, 2p+1), making the program identical across cores/examples
(pure SPMD; all data-dependence lives in host-built input tensors). Each
block's 256 rows span <= W1 source rows (W1 from the data, ~96; a second
K-window W2 is emitted only if some block spans beyond 128). Keeping the
write ring free of loads lets the output stream start as soon as the
first block drains (~13us). bf16 rounding of x gives rel err ~1.6e-3,
well under the 2e-2 gate.
"""

import numpy as np

B, S, D = 16, 4096, 256
NCORES = 8
EX_PER_CORE = B // NCORES  # 2
NK = 256  # out rows per block
CPP = NK // 128  # out rows per partition per block = 2

# pipeline buffer depths
BCB = 8  # bcast sbuf bufs
GN = 10  # G sbuf bufs
PSB = 8  # psum block tensors [128, CPP, D] (1 bank each)
SNP = 6  # staging slot-pairs (12 block slots)
XCH = 8  # x load chunks


def _plan(dimensions):
    d = dimensions[:, :, 0].astype(np.int64)  # [B,S]
    totals = d.sum(1)  # [B]
    T = int(totals.max())
    csum = d.cumsum(1)  # [B,S]
    pos = np.arange(T)
    idx = np.empty((B, T), np.int64)
    for b in range(B):
        idx[b] = np.searchsorted(csum[b], pos, side="right")
    idx = np.minimum(idx, S - 1)
    return T, idx, totals


def build_program(NBE, W1, W2, nk_last, ld_chunk):
    """One SPMD program: NBE NK-row blocks per example, 2 examples.
    ld_chunk[bi] = x-chunk index of block bi. W2=0 means every block's
    source span fits the single W1-row K-window.

    Write units: per example, consecutive block pairs (blk 2j, 2j+1) go out
    as one DMA; an odd final block goes out alone with the nk_last trim.
    Unit u uses staging slot-pair u%SNP (stg block slots 2*(u%SNP)+{0,1}).
    """
    import concourse.bass as bass
    import concourse.bacc as bacc
    import concourse.mybir as mybir
    from concourse import library_config
    from contextlib import ExitStack

    NBLK = EX_PER_CORE * NBE
    R_ex = NBE * NK

    # write units: (blocks, is_last)
    units = []
    for e in range(EX_PER_CORE):
        blk = 0
        while blk < NBE:
            if blk + 1 < NBE:
                units.append((e * NBE + blk, 2))
                blk += 2
            else:
                units.append((e * NBE + blk, 1))
                blk += 1
    unit_of = {}
    slot_of = {}
    for u, (b0, n) in enumerate(units):
        for k in range(n):
            unit_of[b0 + k] = u
            slot_of[b0 + k] = 2 * (u % SNP) + k

    nc = bacc.Bacc("TRN2", num_devices=NCORES, name="expand_pe")
    xw1_t = nc.dram_tensor("xw1", [W1, NBLK, D], mybir.dt.bfloat16, kind="ExternalInput")
    if W2:
        xw2_t = nc.dram_tensor("xw2", [W2, NBLK, D], mybir.dt.bfloat16, kind="ExternalInput")
    idxr_t = nc.dram_tensor("idxr", [1, NBLK, NK], mybir.dt.bfloat16, kind="ExternalInput")
    out_t = nc.dram_tensor(
        "out", [EX_PER_CORE * R_ex, D], mybir.dt.float32, kind="ExternalOutput"
    )

    xsplit = [(i * NBLK) // XCH for i in range(XCH + 1)]

    with (
        nc.sbuf_tensor("xw1_sb", [W1, NBLK, D], mybir.dt.bfloat16) as xw1_sb,
        nc.sbuf_tensor("idxr_sb", [1, NBLK, NK], mybir.dt.bfloat16) as idxr_sb,
        nc.sbuf_tensor("iota_sb", [128, 2], mybir.dt.float32) as iota_sb,
        nc.sbuf_tensor("bcb", [128, BCB, NK], mybir.dt.bfloat16) as bcb,
        nc.sbuf_tensor("gb1", [W1, GN, NK], mybir.dt.bfloat16) as gb1,
        nc.sbuf_tensor("stg", [128, 2 * SNP, CPP, D], mybir.dt.float32) as stg,
        nc.semaphore("ldc") as ldc,
        nc.semaphore("ldc2") as ldc2,
        nc.semaphore("bc") as bc,
        nc.semaphore("cmp") as cmp_s,
        nc.semaphore("mm") as mm,
        nc.semaphore("dr") as dr,
        ExitStack() as stack,
    ):
        if W2:
            xw2_sb = stack.enter_context(
                nc.sbuf_tensor("xw2_sb", [W2, NBLK, D], mybir.dt.bfloat16)
            )
            gb2 = stack.enter_context(
                nc.sbuf_tensor("gb2", [W2, GN, NK], mybir.dt.bfloat16)
            )
        ldx = [stack.enter_context(nc.semaphore(f"ldx{c}")) for c in range(XCH)]  # noqa: ANT232
        wsl = [stack.enter_context(nc.semaphore(f"wsl{s}")) for s in range(SNP)]  # noqa: ANT232
        pst = [
            stack.enter_context(  # noqa: ANT232
                nc.psum_tensor(f"ps{t}", [128, CPP, D], mybir.dt.float32)
            )
            for t in range(PSB)
        ]
        block = stack.enter_context(nc.Block())

        @block.sync
        def _(sy):
            # idxr rides this ring (empty until the first write ~15us);
            # small head first so Pool can start broadcasting early
            sy.dma_start(idxr_sb[:, :16, :], idxr_t.ap()[:, :16, :]).then_inc(ldc, 16)
            sy.dma_start(idxr_sb[:, 16:, :], idxr_t.ap()[:, 16:, :]).then_inc(ldc2, 16)
            for u, (b0, n) in enumerate(units):
                sy.wait_ge(dr, b0 + n)
                r0 = b0 * NK
                if n == 2:
                    dst = out_t.ap()[r0 : r0 + 2 * NK, :].rearrange(
                        "(b p c) e -> p b c e", b=2, p=128
                    )
                    src = stg[:, 2 * (u % SNP) : 2 * (u % SNP) + 2, :, :]
                else:
                    nk = nk_last
                    dst = out_t.ap()[r0 : r0 + nk, :].rearrange(
                        "(p c) e -> p c e", p=nk // CPP
                    )
                    src = stg[: nk // CPP, 2 * (u % SNP), :, :]
                sy.dma_start(dst, src).then_inc(wsl[u % SNP], 16)
            for s in range(SNP):
                nwr = len(units) // SNP + (1 if s < len(units) % SNP else 0)
                sy.wait_ge(wsl[s], 16 * nwr)

        @block.scalar
        def _(sc):
            lo, hi = xsplit[0], xsplit[1]
            sc.dma_start(xw1_sb[:, lo:hi, :], xw1_t.ap()[:, lo:hi, :]).then_inc(ldx[0], 16)
            if W2:
                sc.dma_start(xw2_sb[:, lo:hi, :], xw2_t.ap()[:, lo:hi, :]).then_inc(ldx[0], 16)
            for bi in range(NBLK):
                if bi >= 4 and (bi - 4) % 8 == 0 and (bi - 4) // 8 + 1 < XCH:
                    c = (bi - 4) // 8 + 1
                    clo, chi = xsplit[c], xsplit[c + 1]
                    sc.dma_start(
                        xw1_sb[:, clo:chi, :], xw1_t.ap()[:, clo:chi, :]
                    ).then_inc(ldx[c], 16)
                    if W2:
                        sc.dma_start(
                            xw2_sb[:, clo:chi, :], xw2_t.ap()[:, clo:chi, :]
                        ).then_inc(ldx[c], 16)
                sc.wait_ge(mm, bi + 1)
                u = unit_of[bi]
                if u >= SNP and slot_of[bi] == 2 * (u % SNP):
                    sc.wait_ge(wsl[u % SNP], 16 * (u // SNP))
                sc.copy(stg[:, slot_of[bi], :, :], pst[bi % PSB][:, :, :]).then_inc(
                    dr, 1
                )

        @block.gpsimd
        def _(gp):
            # iota values 0..255 are exact in f32
            gp.iota(
                iota_sb[:],
                [[128, 2]],
                base=0,
                channel_multiplier=1,
                allow_small_or_imprecise_dtypes=True,
            )
            gp.load_library(library_config.mlp)
            gp.wait_ge(ldc, 16)
            for bi in range(NBLK):
                if bi == 16:
                    gp.wait_ge(ldc2, 16)
                if bi >= BCB:
                    gp.wait_ge(cmp_s, bi - BCB + 1)
                gp.partition_broadcast(
                    bcb[: max(W1, W2), bi % BCB, :], idxr_sb[:1, bi, :]
                ).then_inc(bc, 1)

        @block.vector
        def _(ve):
            for bi in range(NBLK):
                ve.wait_ge(bc, bi + 1)
                if bi >= GN:
                    ve.wait_ge(mm, bi - GN + 1)
                c1 = ve.tensor_scalar(
                    gb1[:, bi % GN, :],
                    bcb[:W1, bi % BCB, :],
                    iota_sb[:W1, :1],
                    None,
                    mybir.AluOpType.is_equal,
                )
                if W2:
                    ve.tensor_scalar(
                        gb2[:, bi % GN, :],
                        bcb[:W2, bi % BCB, :],
                        iota_sb[:W2, 1:2],
                        None,
                        mybir.AluOpType.is_equal,
                    ).then_inc(cmp_s, 1)
                else:
                    c1.then_inc(cmp_s, 1)

        @block.tensor
        def _(te):
            for bi in range(NBLK):
                te.wait_ge(cmp_s, bi + 1)
                if bi == 0 or ld_chunk[bi] != ld_chunk[bi - 1]:
                    te.wait_ge(ldx[ld_chunk[bi]], 32 if W2 else 16)
                if bi >= PSB:
                    te.wait_ge(dr, bi - PSB + 1)
                for m in range(CPP):
                    mm1 = te.matmul(
                        pst[bi % PSB][:, m, :],
                        gb1[:, bi % GN, m * 128 : (m + 1) * 128],
                        xw1_sb[:, bi, :],
                        start=True,
                        stop=not W2,
                    )
                    if W2:
                        mm1 = te.matmul(
                            pst[bi % PSB][:, m, :],
                            gb2[:, bi % GN, m * 128 : (m + 1) * 128],
                            xw2_sb[:, bi, :],
                            start=False,
                            stop=True,
                        )
                    if m == CPP - 1:
                        mm1.then_inc(mm, 1)

    nc.compile()
    return nc


def _install_ntff_hook():
    """Provide the antenv.axon_hooks module bass_utils expects for NTFF
    tracing under axon (the agent image ships without it)."""
    import sys
    import types

    if "antenv.axon_hooks" in sys.modules:
        return
    from trn_agent_boot.trn_boot import _ntff_profile_via_ctypes

    hook = _ntff_profile_via_ctypes("/opt/axon/libaxon_pjrt.so")
    mod = types.ModuleType("antenv.axon_hooks")
    state = {"hook": hook}
    mod.get_axon_ntff_profile_hook = lambda: state["hook"]
    mod.set_axon_ntff_profile_hook = lambda h: state.update(hook=h)
    sys.modules["antenv.axon_hooks"] = mod


def kernel(x, dimensions, _trace=False, _sim_core=None):
    import ml_dtypes

    x = np.ascontiguousarray(np.asarray(x), dtype=np.float32)
    dimensions = np.asarray(dimensions).astype(np.int32)

    T, idx, totals = _plan(dimensions)
    NBE = (T + NK - 1) // NK
    R_ex = NBE * NK
    NBLK = EX_PER_CORE * NBE
    nk_last = ((T - (NBE - 1) * NK + CPP - 1) // CPP) * CPP

    # K-window sizes from the data (uniform across the whole batch)
    max_span = 0
    for bb in range(B):
        tot = int(totals[bb])
        for blk in range(NBE):
            c0 = blk * NK
            c1 = min((blk + 1) * NK, tot, T)
            if c0 < c1:
                max_span = max(max_span, int(idx[bb, c1 - 1] - idx[bb, c0] + 1))
    W1 = min(128, max(32, ((max_span + 31) // 32) * 32))
    W2 = max(32, ((max_span - 128 + 31) // 32) * 32) if max_span > 128 else 0
    assert max_span <= 128 + (W2 or 0), f"block span {max_span} exceeds 128+W2"

    xbf = x.astype(ml_dtypes.bfloat16)
    xw1 = np.zeros((B, W1, NBE, D), ml_dtypes.bfloat16)
    xw2 = np.zeros((B, W2, NBE, D), ml_dtypes.bfloat16) if W2 else None
    idxr = np.full((B, NBE, NK), -1.0, np.float32)
    for bb in range(B):
        tot = int(totals[bb])
        for blk in range(NBE):
            c0 = blk * NK
            c1 = min((blk + 1) * NK, tot)
            s = int(idx[bb, c0]) if c0 < c1 else 0
            n1 = min(W1, S - s)
            xw1[bb, :n1, blk] = xbf[bb, s : s + n1]
            if W2:
                n2 = min(W2, S - (s + 128))
                if n2 > 0:
                    xw2[bb, :n2, blk] = xbf[bb, s + 128 : s + 128 + n2]
            if c0 >= c1:
                continue
            t = np.arange(c0, c1)
            q = t - c0
            cols = (q % CPP) * 128 + (q // CPP)
            idxr[bb, blk, cols] = idx[bb, t] - s
    idxr_bf = idxr.astype(ml_dtypes.bfloat16)

    xsplit = [(i * NBLK) // XCH for i in range(XCH + 1)]
    ld_chunk = [
        next(ci for ci in range(XCH) if bi < xsplit[ci + 1]) for bi in range(NBLK)
    ]

    in_maps = []
    for core in range(NCORES):
        exs = [EX_PER_CORE * core + e for e in range(EX_PER_CORE)]
        im = {
            "xw1": np.concatenate([xw1[bb] for bb in exs], axis=1),
            "idxr": np.concatenate([idxr_bf[bb] for bb in exs], axis=0)[None],
        }
        if W2:
            im["xw2"] = np.concatenate([xw2[bb] for bb in exs], axis=1)
        in_maps.append(im)

    nc = build_program(NBE, W1, W2, nk_last, ld_chunk)

    if _sim_core is not None:
        import concourse.bass_interp as bass_interp

        sim = bass_interp.CoreSim(nc)
        for k, v in in_maps[_sim_core].items():
            sim.tensor(k)[:] = v
        sim.simulate()
        st = np.asarray(sim.tensor("out"), dtype=np.float32)
        out = np.empty((EX_PER_CORE, T, D), np.float32)
        for e in range(EX_PER_CORE):
            out[e] = st[e * R_ex : e * R_ex + T]
        return out

    import concourse.bass_utils as bass_utils

    if _trace:
        _install_ntff_hook()
        bass_utils.upload_artifacts = lambda tmpdir: tmpdir

    res = bass_utils.run_bass_kernel_spmd(
        nc, in_maps, core_ids=list(range(NCORES)), trace=_trace
    )

    out = np.empty((B, T, D), np.float32)
    for core in range(NCORES):
        st = res.results[core]["out"]
        for e in range(EX_PER_CORE):
            out[EX_PER_CORE * core + e] = st[e * R_ex : e * R_ex + T]
    if _trace:
        kernel.last_results = res
    return out
